# revision 6
# baseline (speedup 1.0000x reference)
"""GCN 2-layer encoder on 8 TRN2 NeuronCores via Bass/Tile.

Design (measured-primitive driven):
- Edges target-sharded across 8 cores (6250 targets/core).
- Per-edge source rows fetched with gpsimd.dma_gather (~8ns/idx,
  descriptor-bound; the only fast arbitrary-index mover).
- Aggregation: per 128-edge block, one-hot matmul on TensorE into PSUM
  [features, 512-target group]. One-hots are plain 0/1, built with one
  batched tensor_tensor is_equal per gather chunk; all dinv scaling is
  folded into epilogues / host-prescaled tables.
- L1 gathers from xs = dinv-scaled x (bf16; lo/hi split for int16 idx).
- L2 gathers 256B pair-rows of hs2 = dinv*h@W2 (bf16, AllGather'd),
  parity-split one-hot columns.
- Self-loops handled in epilogues (no gather).
"""
import numpy as np
import ml_dtypes

import concourse.bass as bass
import concourse.bacc as bacc
import concourse.mybir as mybir
import concourse.tile as tile
from concourse.bass_utils import run_bass_kernel_spmd
from concourse.library_config import mlp

BF16 = mybir.dt.bfloat16
F32 = mybir.dt.float32
I16 = mybir.dt.int16
NPBF16 = ml_dtypes.bfloat16


def install_ntff_hook():
    import sys, types
    try:
        from antenv.axon_hooks import get_axon_ntff_profile_hook  # noqa
        return
    except ImportError:
        pass
    try:
        from trn_agent_boot import trn_boot
        hook = trn_boot._ntff_profile_via_ctypes("/opt/axon/libaxon_pjrt.so")
    except Exception:
        return
    mod = types.ModuleType("antenv.axon_hooks")
    mod.get_axon_ntff_profile_hook = lambda: hook
    mod.set_axon_ntff_profile_hook = lambda h: None
    sys.modules["antenv.axon_hooks"] = mod


def bcast_mid(ap, n):
    """[P, F] AP -> [P, n, F] with stride-0 middle dim."""
    a = [list(p) for p in ap.ap]
    new = [a[0], [0, n]] + a[1:]
    return bass.AP(ap.tensor, ap.offset, new)


class Cfg:
    def __init__(self, n_nodes=50000, n_cores=8, d_in=128, d_hid=128, d_out=64,
                 win=128, grp_win=4):
        assert n_nodes % (2 * n_cores) == 0
        self.n_nodes = n_nodes
        self.n_cores = n_cores
        self.d_in = d_in
        self.d_hid = d_hid
        self.d_out = d_out
        self.slice = n_nodes // n_cores
        self.win = win
        self.grp_win = grp_win
        self.grp = win * grp_win
        self.ngrp = -(-self.slice // self.grp)
        self.nwin = self.ngrp * grp_win          # padded window count
        self.slice_pad = self.ngrp * self.grp
        self.lo_cut = min(32768, n_nodes)
        self.pairs = n_nodes // 2


def _wrap_idx(idx):
    idx = np.asarray(idx, np.int16)
    w = idx.reshape(-1, 16).T
    return np.tile(w, (8, 1)).astype(np.int16)


def _interleave_cols(vals):
    v = np.asarray(vals)
    return np.ascontiguousarray(v.reshape(-1, 128).T)


class EdgePlan:
    def __init__(self, cfg, per_core_edges, idx_of, col_of, force_min=False):
        self.cfg = cfg
        counts = np.zeros((cfg.n_cores, cfg.nwin), np.int64)
        for c in range(cfg.n_cores):
            w = per_core_edges[c]["win"]
            if len(w):
                np.add.at(counts[c], w, 1)
        bpw = -(-counts.max(axis=0) // 128)
        if force_min:
            bpw = np.maximum(bpw, 1)
        self.bpw = bpw.astype(np.int64)
        self.nblk = int(bpw.sum())
        self.idx = []
        self.col = []
        for c in range(cfg.n_cores):
            e = per_core_edges[c]
            order = np.argsort(e["win"], kind="stable")
            win_s = e["win"][order]
            idx_s = idx_of(e, order)
            col_s = col_of(e, order)
            idx_out = np.zeros(self.nblk * 128, np.int16)
            col_out = np.full(self.nblk * 128, -99.0, np.float32)
            starts = np.searchsorted(win_s, np.arange(cfg.nwin))
            ends = np.searchsorted(win_s, np.arange(cfg.nwin) + 1)
            off = 0
            for w in range(cfg.nwin):
                n = int(ends[w] - starts[w])
                cap = int(self.bpw[w]) * 128
                assert n <= cap
                idx_out[off:off + n] = idx_s[starts[w]:ends[w]]
                col_out[off:off + n] = col_s[starts[w]:ends[w]]
                off += cap
            self.idx.append(_wrap_idx(idx_out))
            self.col.append(_interleave_cols(col_out).astype(NPBF16))
        self.blocks = []
        for w in range(cfg.nwin):
            g = w // cfg.grp_win
            woff = w % cfg.grp_win
            for b in range(int(self.bpw[w])):
                self.blocks.append(
                    (g, woff, b == 0, b == int(self.bpw[w]) - 1))


def host_prep(cfg, x, edge_index, W1, b1, W2, b2):
    N = cfg.n_nodes
    src = np.asarray(edge_index[0], np.int64)
    tgt = np.asarray(edge_index[1], np.int64)
    deg = np.bincount(tgt, minlength=N).astype(np.float64) + 1.0
    dinv = (1.0 / np.sqrt(deg)).astype(np.float32)
    xs = (np.asarray(x, np.float32) * dinv[:, None]).astype(NPBF16)

    core = tgt // cfg.slice
    tloc = tgt % cfg.slice
    win = tloc // cfg.win
    ww = tloc % cfg.win

    per_core, lo, hi = [], [], []
    for c in range(cfg.n_cores):
        m = core == c
        e = {"src": src[m], "win": win[m], "ww": ww[m]}
        per_core.append(e)
        ml = e["src"] < cfg.lo_cut
        lo.append({"src": e["src"][ml], "win": e["win"][ml], "ww": e["ww"][ml]})
        mh = ~ml
        hi.append({"src": e["src"][mh] - cfg.lo_cut, "win": e["win"][mh],
                   "ww": e["ww"][mh]})

    plan_lo = EdgePlan(cfg, lo, lambda e, o: e["src"][o].astype(np.int16),
                       lambda e, o: e["ww"][o].astype(np.float32))
    plan_hi = EdgePlan(cfg, hi, lambda e, o: e["src"][o].astype(np.int16),
                       lambda e, o: e["ww"][o].astype(np.float32))
    l2loc, l2rem = [], []
    for c in range(cfg.n_cores):
        e = per_core[c]
        mloc = (e["src"] // cfg.slice) == c
        l2loc.append({
            "pidx": ((e["src"][mloc] - c * cfg.slice) >> 1).astype(np.int16),
            "win": e["win"][mloc],
            "col": (e["ww"][mloc] + cfg.win * (e["src"][mloc] & 1)).astype(np.float32),
        })
        mrem = ~mloc
        l2rem.append({
            "pidx": (e["src"][mrem] >> 1).astype(np.int16),
            "win": e["win"][mrem],
            "col": (e["ww"][mrem] + cfg.win * (e["src"][mrem] & 1)).astype(np.float32),
        })
    pick_idx = lambda e, o: e["pidx"][o]
    pick_col = lambda e, o: e["col"][o]
    plan_l2l = EdgePlan(cfg, l2loc, pick_idx, pick_col)
    plan_l2r = EdgePlan(cfg, l2rem, pick_idx, pick_col)

    shared = {
        "xs_lo": np.ascontiguousarray(xs[:cfg.lo_cut]),
        "W1": np.asarray(W1, np.float32).astype(NPBF16),
        "W2": np.asarray(W2, np.float32).astype(NPBF16),
        "iota1": np.tile(np.arange(cfg.win, dtype=np.float32),
                         (128, 1)).astype(NPBF16),
        "iota2": np.tile(np.arange(2 * cfg.win, dtype=np.float32),
                         (128, 1)).astype(NPBF16),
        "ident": np.eye(cfg.d_out, dtype=np.float32).astype(NPBF16),
    }
    if cfg.n_nodes > cfg.lo_cut:
        shared["xs_hi"] = np.ascontiguousarray(xs[cfg.lo_cut:])
    has_b1 = bool(np.any(np.asarray(b1)))
    has_b2 = bool(np.any(np.asarray(b2)))
    if has_b2:
        shared["b2col"] = np.asarray(b2, np.float32).reshape(cfg.d_out, 1)
    in_maps = []
    for c in range(cfg.n_cores):
        s0 = c * cfg.slice
        dpad = np.pad(dinv[s0:s0 + cfg.slice], (0, cfg.slice_pad - cfg.slice))
        selfT = np.zeros((cfg.d_in, cfg.slice_pad), np.float32)
        selfT[:, :cfg.slice] = xs[s0:s0 + cfg.slice].astype(np.float32).T
        m = {
            "idx_lo": plan_lo.idx[c], "col_lo": plan_lo.col[c],
            "idx_l2l": plan_l2l.idx[c], "col_l2l": plan_l2l.col[c],
            "idx_l2r": plan_l2r.idx[c], "col_l2r": plan_l2r.col[c],
            "selfT": selfT.astype(NPBF16),
            "dinv2T": np.tile(dpad * dpad, (cfg.d_hid, 1)).astype(NPBF16),
            "dinvT64": np.tile(dpad, (cfg.d_out, 1)).astype(NPBF16),
        }
        if cfg.n_nodes > cfg.lo_cut:
            m["idx_hi"] = plan_hi.idx[c]
            m["col_hi"] = plan_hi.col[c]
        if has_b1:
            m["b1dT"] = np.outer(np.asarray(b1, np.float32), dpad).astype(NPBF16)
        m.update(shared)
        in_maps.append(m)
    return plan_lo, plan_hi, (plan_l2l, plan_l2r), in_maps, has_b1, has_b2


def build(cfg, plan_lo, plan_hi, plan_l2, has_b1, has_b2):
    plan_l2l, plan_l2r = plan_l2
    nc = bacc.Bacc("TRN2", num_devices=cfg.n_cores)
    D, DH, DO = cfg.d_in, cfg.d_hid, cfg.d_out
    SP = cfg.slice_pad
    G = cfg.grp
    W = cfg.win
    has_hi = cfg.n_nodes > cfg.lo_cut

    t_xs_lo = nc.dram_tensor("xs_lo", [cfg.lo_cut, D], BF16, kind="ExternalInput")
    if has_hi:
        t_xs_hi = nc.dram_tensor("xs_hi", [cfg.n_nodes - cfg.lo_cut, D], BF16,
                                 kind="ExternalInput")
    t_W1 = nc.dram_tensor("W1", [D, DH], BF16, kind="ExternalInput")
    t_W2 = nc.dram_tensor("W2", [DH, DO], BF16, kind="ExternalInput")
    t_io1 = nc.dram_tensor("iota1", [128, W], BF16, kind="ExternalInput")
    t_io2 = nc.dram_tensor("iota2", [128, 2 * W], BF16, kind="ExternalInput")
    t_id = nc.dram_tensor("ident", [DO, DO], BF16, kind="ExternalInput")
    t_selfT = nc.dram_tensor("selfT", [D, SP], BF16, kind="ExternalInput")
    t_d2T = nc.dram_tensor("dinv2T", [DH, SP], BF16, kind="ExternalInput")
    t_dT64 = nc.dram_tensor("dinvT64", [DO, SP], BF16, kind="ExternalInput")
    t_b1dT = (nc.dram_tensor("b1dT", [DH, SP], BF16, kind="ExternalInput")
              if has_b1 else None)
    t_b2 = (nc.dram_tensor("b2col", [DO, 1], F32, kind="ExternalInput")
            if has_b2 else None)
    t_ilo = nc.dram_tensor("idx_lo", list(plan_lo.idx[0].shape), I16,
                           kind="ExternalInput")
    t_clo = nc.dram_tensor("col_lo", list(plan_lo.col[0].shape), BF16,
                           kind="ExternalInput")
    if has_hi:
        t_ihi = nc.dram_tensor("idx_hi", list(plan_hi.idx[0].shape), I16,
                               kind="ExternalInput")
        t_chi = nc.dram_tensor("col_hi", list(plan_hi.col[0].shape), BF16,
                               kind="ExternalInput")
    t_il2l = nc.dram_tensor("idx_l2l", list(plan_l2l.idx[0].shape), I16,
                            kind="ExternalInput")
    t_cl2l = nc.dram_tensor("col_l2l", list(plan_l2l.col[0].shape), BF16,
                            kind="ExternalInput")
    t_il2r = nc.dram_tensor("idx_l2r", list(plan_l2r.idx[0].shape), I16,
                            kind="ExternalInput")
    t_cl2r = nc.dram_tensor("col_l2r", list(plan_l2r.col[0].shape), BF16,
                            kind="ExternalInput")
    t_out = nc.dram_tensor("out", [DO, cfg.slice], F32, kind="ExternalOutput")
    import os as _os
    dbg = bool(_os.environ.get("GCN_DBG"))
    t_dbg = (nc.dram_tensor("dbg", [D, SP], F32, kind="ExternalOutput")
             if dbg else None)

    cc_in = nc.dram_tensor("cc_in", [cfg.slice, DO], BF16)
    cc_out = nc.dram_tensor("cc_out", [cfg.n_cores, cfg.slice, DO], BF16,
                            addr_space="Shared")

    def group_blocks(plan):
        out = [[] for _ in range(cfg.ngrp)]
        for i, blk in enumerate(plan.blocks):
            out[blk[0]].append((i,) + blk[1:])
        return out

    gb_lo, gb_hi = map(group_blocks, (plan_lo, plan_hi))
    gb_l2l, gb_l2r = map(group_blocks, (plan_l2l, plan_l2r))

    with tile.TileContext(nc) as tc:
        with (
            tc.tile_pool(name="meta", bufs=1) as meta,
            tc.tile_pool(name="gath", bufs=3) as gpool,
            tc.tile_pool(name="work", bufs=3) as pool,
            tc.tile_pool(name="res", bufs=1) as res,
            tc.tile_pool(name="psA", bufs=4, space="PSUM") as psA,
            tc.tile_pool(name="psB", bufs=1, space="PSUM") as psB,
        ):
            nc.gpsimd.load_library(mlp)

            def load(t):
                s = meta.tile(list(t.shape), t.dtype, tag=f"m_{t.name}")
                nc.sync.dma_start(s[:], t[:])
                return s

            ilo, clo = load(t_ilo), load(t_clo)
            ihi, chi = (load(t_ihi), load(t_chi)) if has_hi else (None, None)
            il2l, cl2l = load(t_il2l), load(t_cl2l)
            il2r, cl2r = load(t_il2r), load(t_cl2r)
            io1, io2, idn = load(t_io1), load(t_io2), load(t_id)
            W1s, W2s = load(t_W1), load(t_W2)
            selfT, d2T, dT64 = load(t_selfT), load(t_d2T), load(t_dT64)
            b1dT = load(t_b1dT) if has_b1 else None
            b2s = load(t_b2) if has_b2 else None

            zeros = res.tile([128, cfg.grp], BF16)
            nc.gpsimd.memset(zeros[:], 0.0)
            hs2T = res.tile([DO, SP], BF16)
            ccstage = res.tile([128, SP // 128, DO], BF16)

            def agg_layer(g, plans, psum_tile, m_rows):
                nc.tensor.matmul(psum_tile[:], zeros[:, 0:128], zeros[:],
                                 start=True, stop=False, skip_group_check=True)
                last_pass = {}
                for pi, (plan, gb, *_r) in enumerate(plans):
                    for (i, woff, first, last) in gb[g]:
                        if last:
                            last_pass[woff] = pi
                for pi, (plan, gb, idx_t, col_t, src_ap, iota_t, psplit) \
                        in enumerate(plans):
                    all_blocks = gb[g]
                    if not all_blocks:
                        continue
                    scols = 2 * W if psplit else W
                    CH = 32
                    for c0 in range(0, len(all_blocks), CH):
                        blocks = all_blocks[c0:c0 + CH]
                        i0 = blocks[0][0]
                        nblk = len(blocks)
                        nidx = nblk * 128
                        gt = gpool.tile([128, nblk, 128], BF16, tag="g")
                        nc.gpsimd.dma_gather(
                            gt[:, 0:nblk, :], src_ap,
                            idx_t[:, i0 * 8: (i0 + nblk) * 8],
                            nidx, nidx, 128, single_packet=False)
                        St = gpool.tile([128, nblk, scols], BF16, tag="S")
                        nc.vector.tensor_tensor(
                            St[:, 0:nblk, :], bcast_mid(iota_t[:], nblk),
                            col_t[:, i0:i0 + nblk].to_broadcast(
                                [128, nblk, scols]),
                            mybir.AluOpType.is_equal)
                        _emit(blocks, gt, St, psum_tile, last_pass, pi, psplit)

            def _emit(blocks, gt, St, psum_tile, last_pass, pi, psplit):
                if True:
                    for bi, (i, woff, first, last) in enumerate(blocks):
                        st = False
                        sp = last and last_pass.get(woff, pi) == pi
                        if psplit:
                            seg = psum_tile[0:DO, woff * W:(woff + 1) * W]
                            nc.tensor.matmul(
                                seg, gt[:, bi, 0:DO], St[:, bi, 0:W],
                                start=st, stop=False, skip_group_check=True)
                            nc.tensor.matmul(
                                seg, gt[:, bi, DO:2 * DO], St[:, bi, W:2 * W],
                                start=False, stop=sp, skip_group_check=True)
                        else:
                            seg = psum_tile[:, woff * W:(woff + 1) * W]
                            nc.tensor.matmul(
                                seg, gt[:, bi, :], St[:, bi, :],
                                start=st, stop=sp, skip_group_check=True)

            import os
            l1_plans = [(plan_lo, gb_lo, ilo, clo, t_xs_lo.ap(), io1, False)]
            if has_hi and not os.environ.get("GCN_NO_HI"):
                l1_plans.append((plan_hi, gb_hi, ihi, chi, t_xs_hi.ap(), io1, False))

            # ---------------- Layer 1 + transform + hs2 ----------------
            for g in range(cfg.ngrp):
                agg = psA.tile([128, G], F32, tag="agg")
                agg_layer(g, l1_plans, agg, D)
                sl = slice(g * G, (g + 1) * G)
                aggu = pool.tile([D, G], BF16, tag="aggu")
                nc.vector.tensor_tensor(aggu[:], agg[:], selfT[:, sl],
                                        mybir.AluOpType.add)
                if dbg:
                    dglt = pool.tile([D, G], F32, tag="dbgt")
                    nc.vector.tensor_copy(dglt[:], agg[:])
                    nc.sync.dma_start(t_dbg[:, sl], dglt[:])
                xf = psB.tile([DH, G], F32, tag="xf")
                nc.tensor.matmul(xf[:], W1s[:], aggu[:], start=True, stop=True)
                tm = pool.tile([DH, G], BF16, tag="tm")
                nc.vector.tensor_tensor(tm[:], xf[:], d2T[:, sl],
                                        mybir.AluOpType.mult)
                if has_b1:
                    nc.vector.tensor_tensor(tm[:], tm[:], b1dT[:, sl],
                                            mybir.AluOpType.add)
                hsg = pool.tile([DH, G], BF16, tag="hs")
                nc.scalar.activation(hsg[:], tm[:],
                                     mybir.ActivationFunctionType.Relu)
                xf2 = psB.tile([DO, G], F32, tag="xf2")
                nc.tensor.matmul(xf2[:], W2s[:], hsg[:], start=True, stop=True)
                nc.vector.tensor_copy(hs2T[:, sl], xf2[:])
                for q in range(G // 128):
                    tp = psB.tile([128, DO], BF16, tag="tp")
                    nc.tensor.transpose(
                        tp[:], hs2T[:, g * G + q * 128: g * G + (q + 1) * 128],
                        idn[:])
                    nc.scalar.activation(ccstage[:, g * (G // 128) + q, :],
                                         tp[:],
                                         mybir.ActivationFunctionType.Copy)

            # ---------------- exchange ----------------
            nfull = cfg.slice // 128
            rem = cfg.slice - nfull * 128
            flat = cc_in.ap()
            nc.sync.dma_start(
                flat[0:nfull * 128, :].rearrange("(a p) f -> p a f", p=128),
                ccstage[:, 0:nfull, :])
            if rem:
                nc.sync.dma_start(flat[nfull * 128:cfg.slice, :],
                                  ccstage[0:rem, nfull, :])
            nc.gpsimd.collective_compute(
                "AllGather", mybir.AluOpType.bypass,
                replica_groups=[list(range(cfg.n_cores))],
                ins=[cc_in.ap().opt()], outs=[cc_out.ap().opt()])

            # ---------------- Layer 2 ----------------
            l2src = cc_out.ap().rearrange("c (a two) d -> (c a) (two d)", two=2)
            l2loc_src = cc_in.ap().rearrange("(a two) d -> a (two d)", two=2)
            for g in range(cfg.ngrp):
                agg2 = psA.tile([128, G], F32, tag="agg")
                agg_layer(g,
                          [(plan_l2l, gb_l2l, il2l, cl2l, l2loc_src, io2, True),
                           (plan_l2r, gb_l2r, il2r, cl2r, l2src, io2, True)],
                          agg2, 2 * DO)
                sl = slice(g * G, (g + 1) * G)
                u = pool.tile([DO, G], BF16, tag="u")
                nc.vector.tensor_tensor(u[:], agg2[0:DO, :], hs2T[:, sl],
                                        mybir.AluOpType.add)
                u2 = pool.tile([DO, G], F32, tag="u2")
                nc.vector.tensor_tensor(u2[:], u[:], dT64[:, sl],
                                        mybir.AluOpType.mult)
                if has_b2:
                    nc.vector.tensor_scalar(
                        u2[:], u2[:], b2s[:, 0:1], None,
                        mybir.AluOpType.add)
                ncols = min(cfg.slice, (g + 1) * G) - g * G
                if ncols > 0:
                    nc.sync.dma_start(
                        t_out[:, g * G: g * G + ncols], u2[:, 0:ncols])
    nc.compile()
    return nc


def run(x, edge_index, W1, b1, W2, b2, trace=False, cfg=None):
    install_ntff_hook()
    cfg = cfg or Cfg()
    plan_lo, plan_hi, plan_l2, in_maps, has_b1, has_b2 = host_prep(
        cfg, x, edge_index, W1, b1, W2, b2)
    nc = build(cfg, plan_lo, plan_hi, plan_l2, has_b1, has_b2)
    res = run_bass_kernel_spmd(nc, in_maps, core_ids=list(range(cfg.n_cores)),
                               trace=trace)
    out = np.empty((cfg.n_nodes, cfg.d_out), np.float32)
    for c in range(cfg.n_cores):
        out[c * cfg.slice:(c + 1) * cfg.slice] = \
            np.asarray(res.results[c]["out"], np.float32).T
    return out, res


def kernel(x, edge_index, W1, b1, W2, b2):
    out, _ = run(x, edge_index, W1, b1, W2, b2, trace=False)
    return out


# revision 7
# speedup vs baseline: 1.0206x; 1.0206x over previous
"""GCN 2-layer encoder on 8 TRN2 NeuronCores via Bass/Tile.

Design (measured-primitive driven):
- Edges target-sharded across 8 cores (6250 targets/core).
- Per-edge source rows fetched with gpsimd.dma_gather (~8ns/idx,
  descriptor-bound; the only fast arbitrary-index mover).
- Aggregation: per 128-edge block, one-hot matmul on TensorE into PSUM
  [features, 512-target group]. One-hots are plain 0/1, built with one
  batched tensor_tensor is_equal per gather chunk; all dinv scaling is
  folded into epilogues / host-prescaled tables.
- L1 gathers from xs = dinv-scaled x (bf16; lo/hi split for int16 idx).
- L2 gathers 256B pair-rows of hs2 = dinv*h@W2 (bf16, AllGather'd),
  parity-split one-hot columns.
- Self-loops handled in epilogues (no gather).
"""
import numpy as np
import ml_dtypes

import concourse.bass as bass
import concourse.bacc as bacc
import concourse.mybir as mybir
import concourse.tile as tile
from concourse.bass_utils import run_bass_kernel_spmd
from concourse.library_config import mlp

BF16 = mybir.dt.bfloat16
F32 = mybir.dt.float32
I16 = mybir.dt.int16
NPBF16 = ml_dtypes.bfloat16


def install_ntff_hook():
    import sys, types
    try:
        from antenv.axon_hooks import get_axon_ntff_profile_hook  # noqa
        return
    except ImportError:
        pass
    try:
        from trn_agent_boot import trn_boot
        hook = trn_boot._ntff_profile_via_ctypes("/opt/axon/libaxon_pjrt.so")
    except Exception:
        return
    mod = types.ModuleType("antenv.axon_hooks")
    mod.get_axon_ntff_profile_hook = lambda: hook
    mod.set_axon_ntff_profile_hook = lambda h: None
    sys.modules["antenv.axon_hooks"] = mod


def bcast_mid(ap, n):
    """[P, F] AP -> [P, n, F] with stride-0 middle dim."""
    a = [list(p) for p in ap.ap]
    new = [a[0], [0, n]] + a[1:]
    return bass.AP(ap.tensor, ap.offset, new)


class Cfg:
    def __init__(self, n_nodes=50000, n_cores=8, d_in=128, d_hid=128, d_out=64,
                 win=128, grp_win=4):
        assert n_nodes % (2 * n_cores) == 0
        self.n_nodes = n_nodes
        self.n_cores = n_cores
        self.d_in = d_in
        self.d_hid = d_hid
        self.d_out = d_out
        self.slice = n_nodes // n_cores
        self.win = win
        self.grp_win = grp_win
        self.grp = win * grp_win
        self.ngrp = -(-self.slice // self.grp)
        self.nwin = self.ngrp * grp_win          # padded window count
        self.slice_pad = self.ngrp * self.grp
        self.lo_cut = min(32768, n_nodes)
        self.pairs = n_nodes // 2


def _wrap_idx(idx):
    idx = np.asarray(idx, np.int16)
    w = idx.reshape(-1, 16).T
    return np.tile(w, (8, 1)).astype(np.int16)


def _interleave_cols(vals):
    v = np.asarray(vals)
    return np.ascontiguousarray(v.reshape(-1, 128).T)


class EdgePlan:
    def __init__(self, cfg, per_core_edges, idx_of, col_of, force_min=False):
        self.cfg = cfg
        counts = np.zeros((cfg.n_cores, cfg.nwin), np.int64)
        for c in range(cfg.n_cores):
            w = per_core_edges[c]["win"]
            if len(w):
                np.add.at(counts[c], w, 1)
        bpw = -(-counts.max(axis=0) // 128)
        if force_min:
            bpw = np.maximum(bpw, 1)
        self.bpw = bpw.astype(np.int64)
        self.nblk = int(bpw.sum())
        self.idx = []
        self.col = []
        for c in range(cfg.n_cores):
            e = per_core_edges[c]
            order = np.argsort(e["win"], kind="stable")
            win_s = e["win"][order]
            idx_s = idx_of(e, order)
            col_s = col_of(e, order)
            idx_out = np.zeros(self.nblk * 128, np.int16)
            col_out = np.full(self.nblk * 128, -99.0, np.float32)
            starts = np.searchsorted(win_s, np.arange(cfg.nwin))
            ends = np.searchsorted(win_s, np.arange(cfg.nwin) + 1)
            off = 0
            for w in range(cfg.nwin):
                n = int(ends[w] - starts[w])
                cap = int(self.bpw[w]) * 128
                assert n <= cap
                idx_out[off:off + n] = idx_s[starts[w]:ends[w]]
                col_out[off:off + n] = col_s[starts[w]:ends[w]]
                off += cap
            self.idx.append(_wrap_idx(idx_out))
            self.col.append(_interleave_cols(col_out).astype(NPBF16))
        self.blocks = []
        for w in range(cfg.nwin):
            g = w // cfg.grp_win
            woff = w % cfg.grp_win
            for b in range(int(self.bpw[w])):
                self.blocks.append(
                    (g, woff, b == 0, b == int(self.bpw[w]) - 1))


def host_prep(cfg, x, edge_index, W1, b1, W2, b2):
    N = cfg.n_nodes
    src = np.asarray(edge_index[0], np.int64)
    tgt = np.asarray(edge_index[1], np.int64)
    deg = np.bincount(tgt, minlength=N).astype(np.float64) + 1.0
    dinv = (1.0 / np.sqrt(deg)).astype(np.float32)
    xs = (np.asarray(x, np.float32) * dinv[:, None]).astype(NPBF16)

    core = tgt // cfg.slice
    tloc = tgt % cfg.slice
    win = tloc // cfg.win
    ww = tloc % cfg.win

    per_core, lo, hi = [], [], []
    for c in range(cfg.n_cores):
        m = core == c
        e = {"src": src[m], "win": win[m], "ww": ww[m]}
        per_core.append(e)
        ml = e["src"] < cfg.lo_cut
        lo.append({"src": e["src"][ml], "win": e["win"][ml], "ww": e["ww"][ml]})
        mh = ~ml
        hi.append({"src": e["src"][mh] - cfg.lo_cut, "win": e["win"][mh],
                   "ww": e["ww"][mh]})

    plan_lo = EdgePlan(cfg, lo, lambda e, o: e["src"][o].astype(np.int16),
                       lambda e, o: e["ww"][o].astype(np.float32))
    plan_hi = EdgePlan(cfg, hi, lambda e, o: e["src"][o].astype(np.int16),
                       lambda e, o: e["ww"][o].astype(np.float32))
    l2loc, l2rem = [], []
    for c in range(cfg.n_cores):
        e = per_core[c]
        mloc = (e["src"] // cfg.slice) == c
        l2loc.append({
            "pidx": ((e["src"][mloc] - c * cfg.slice) >> 1).astype(np.int16),
            "win": e["win"][mloc],
            "col": (e["ww"][mloc] + cfg.win * (e["src"][mloc] & 1)).astype(np.float32),
        })
        mrem = ~mloc
        l2rem.append({
            "pidx": (e["src"][mrem] >> 1).astype(np.int16),
            "win": e["win"][mrem],
            "col": (e["ww"][mrem] + cfg.win * (e["src"][mrem] & 1)).astype(np.float32),
        })
    pick_idx = lambda e, o: e["pidx"][o]
    pick_col = lambda e, o: e["col"][o]
    plan_l2l = EdgePlan(cfg, l2loc, pick_idx, pick_col)
    plan_l2r = EdgePlan(cfg, l2rem, pick_idx, pick_col)

    shared = {
        "xs_lo": np.ascontiguousarray(xs[:cfg.lo_cut]),
        "W1": np.asarray(W1, np.float32).astype(NPBF16),
        "W2": np.asarray(W2, np.float32).astype(NPBF16),
        "iota1": np.tile(np.arange(cfg.win, dtype=np.float32),
                         (128, 1)).astype(NPBF16),
        "iota2": np.tile(np.arange(2 * cfg.win, dtype=np.float32),
                         (128, 1)).astype(NPBF16),
        "ident": np.eye(cfg.d_out, dtype=np.float32).astype(NPBF16),
    }
    if cfg.n_nodes > cfg.lo_cut:
        shared["xs_hi"] = np.ascontiguousarray(xs[cfg.lo_cut:])
    has_b1 = bool(np.any(np.asarray(b1)))
    has_b2 = bool(np.any(np.asarray(b2)))
    if has_b2:
        shared["b2col"] = np.asarray(b2, np.float32).reshape(cfg.d_out, 1)
    in_maps = []
    for c in range(cfg.n_cores):
        s0 = c * cfg.slice
        dpad = np.pad(dinv[s0:s0 + cfg.slice], (0, cfg.slice_pad - cfg.slice))
        selfT = np.zeros((cfg.d_in, cfg.slice_pad), np.float32)
        selfT[:, :cfg.slice] = xs[s0:s0 + cfg.slice].astype(np.float32).T
        m = {
            "idx_lo": plan_lo.idx[c], "col_lo": plan_lo.col[c],
            "idx_l2l": plan_l2l.idx[c], "col_l2l": plan_l2l.col[c],
            "idx_l2r": plan_l2r.idx[c], "col_l2r": plan_l2r.col[c],
            "selfT": selfT.astype(NPBF16),
            "dinv2T": np.tile(dpad * dpad, (cfg.d_hid, 1)).astype(NPBF16),
            "dinvT64": np.tile(dpad, (cfg.d_out, 1)).astype(NPBF16),
        }
        if cfg.n_nodes > cfg.lo_cut:
            m["idx_hi"] = plan_hi.idx[c]
            m["col_hi"] = plan_hi.col[c]
        if has_b1:
            m["b1dT"] = np.outer(np.asarray(b1, np.float32), dpad).astype(NPBF16)
        m.update(shared)
        in_maps.append(m)
    return plan_lo, plan_hi, (plan_l2l, plan_l2r), in_maps, has_b1, has_b2


def build(cfg, plan_lo, plan_hi, plan_l2, has_b1, has_b2):
    plan_l2l, plan_l2r = plan_l2
    nc = bacc.Bacc("TRN2", num_devices=cfg.n_cores)
    D, DH, DO = cfg.d_in, cfg.d_hid, cfg.d_out
    SP = cfg.slice_pad
    G = cfg.grp
    W = cfg.win
    has_hi = cfg.n_nodes > cfg.lo_cut

    t_xs_lo = nc.dram_tensor("xs_lo", [cfg.lo_cut, D], BF16, kind="ExternalInput")
    if has_hi:
        t_xs_hi = nc.dram_tensor("xs_hi", [cfg.n_nodes - cfg.lo_cut, D], BF16,
                                 kind="ExternalInput")
    t_W1 = nc.dram_tensor("W1", [D, DH], BF16, kind="ExternalInput")
    t_W2 = nc.dram_tensor("W2", [DH, DO], BF16, kind="ExternalInput")
    t_io1 = nc.dram_tensor("iota1", [128, W], BF16, kind="ExternalInput")
    t_io2 = nc.dram_tensor("iota2", [128, 2 * W], BF16, kind="ExternalInput")
    t_id = nc.dram_tensor("ident", [DO, DO], BF16, kind="ExternalInput")
    t_selfT = nc.dram_tensor("selfT", [D, SP], BF16, kind="ExternalInput")
    t_d2T = nc.dram_tensor("dinv2T", [DH, SP], BF16, kind="ExternalInput")
    t_dT64 = nc.dram_tensor("dinvT64", [DO, SP], BF16, kind="ExternalInput")
    t_b1dT = (nc.dram_tensor("b1dT", [DH, SP], BF16, kind="ExternalInput")
              if has_b1 else None)
    t_b2 = (nc.dram_tensor("b2col", [DO, 1], F32, kind="ExternalInput")
            if has_b2 else None)
    t_ilo = nc.dram_tensor("idx_lo", list(plan_lo.idx[0].shape), I16,
                           kind="ExternalInput")
    t_clo = nc.dram_tensor("col_lo", list(plan_lo.col[0].shape), BF16,
                           kind="ExternalInput")
    if has_hi:
        t_ihi = nc.dram_tensor("idx_hi", list(plan_hi.idx[0].shape), I16,
                               kind="ExternalInput")
        t_chi = nc.dram_tensor("col_hi", list(plan_hi.col[0].shape), BF16,
                               kind="ExternalInput")
    t_il2l = nc.dram_tensor("idx_l2l", list(plan_l2l.idx[0].shape), I16,
                            kind="ExternalInput")
    t_cl2l = nc.dram_tensor("col_l2l", list(plan_l2l.col[0].shape), BF16,
                            kind="ExternalInput")
    t_il2r = nc.dram_tensor("idx_l2r", list(plan_l2r.idx[0].shape), I16,
                            kind="ExternalInput")
    t_cl2r = nc.dram_tensor("col_l2r", list(plan_l2r.col[0].shape), BF16,
                            kind="ExternalInput")
    t_out = nc.dram_tensor("out", [DO, cfg.slice], F32, kind="ExternalOutput")
    import os as _os
    dbg = bool(_os.environ.get("GCN_DBG"))
    t_dbg = (nc.dram_tensor("dbg", [D, SP], F32, kind="ExternalOutput")
             if dbg else None)

    cc_in = nc.dram_tensor("cc_in", [cfg.slice, DO], BF16)
    cc_out = nc.dram_tensor("cc_out", [cfg.n_cores, cfg.slice, DO], BF16,
                            addr_space="Shared")

    def group_blocks(plan):
        out = [[] for _ in range(cfg.ngrp)]
        for i, blk in enumerate(plan.blocks):
            out[blk[0]].append((i,) + blk[1:])
        return out

    gb_lo, gb_hi = map(group_blocks, (plan_lo, plan_hi))
    gb_l2l, gb_l2r = map(group_blocks, (plan_l2l, plan_l2r))

    with tile.TileContext(nc) as tc:
        with (
            tc.tile_pool(name="meta", bufs=1) as meta,
            tc.tile_pool(name="gath", bufs=3) as gpool,
            tc.tile_pool(name="work", bufs=3) as pool,
            tc.tile_pool(name="res", bufs=1) as res,
            tc.tile_pool(name="psA", bufs=4, space="PSUM") as psA,
            tc.tile_pool(name="psB", bufs=1, space="PSUM") as psB,
        ):
            nc.gpsimd.load_library(mlp)

            def load(t):
                s = meta.tile(list(t.shape), t.dtype, tag=f"m_{t.name}")
                nc.sync.dma_start(s[:], t[:])
                return s

            ilo, clo = load(t_ilo), load(t_clo)
            ihi, chi = (load(t_ihi), load(t_chi)) if has_hi else (None, None)
            il2l, cl2l = load(t_il2l), load(t_cl2l)
            il2r, cl2r = load(t_il2r), load(t_cl2r)
            io1, io2, idn = load(t_io1), load(t_io2), load(t_id)
            W1s, W2s = load(t_W1), load(t_W2)
            selfT, d2T, dT64 = load(t_selfT), load(t_d2T), load(t_dT64)
            b1dT = load(t_b1dT) if has_b1 else None
            b2s = load(t_b2) if has_b2 else None

            zeros = res.tile([128, cfg.grp], BF16)
            nc.gpsimd.memset(zeros[:], 0.0)
            hs2T = res.tile([DO, SP], BF16)
            locacc = res.tile([DO, SP], BF16)
            ccstage = res.tile([128, SP // 128, DO], BF16)

            def agg_layer(g, plans, psum_tile, m_rows):
                nc.tensor.matmul(psum_tile[:], zeros[:, 0:128], zeros[:],
                                 start=True, stop=False, skip_group_check=True)
                last_pass = {}
                for pi, (plan, gb, *_r) in enumerate(plans):
                    for (i, woff, first, last) in gb[g]:
                        if last:
                            last_pass[woff] = pi
                for pi, (plan, gb, idx_t, col_t, src_ap, iota_t, psplit) \
                        in enumerate(plans):
                    all_blocks = gb[g]
                    if not all_blocks:
                        continue
                    scols = 2 * W if psplit else W
                    CH = 32
                    for c0 in range(0, len(all_blocks), CH):
                        blocks = all_blocks[c0:c0 + CH]
                        i0 = blocks[0][0]
                        nblk = len(blocks)
                        nidx = nblk * 128
                        gt = gpool.tile([128, nblk, 128], BF16, tag="g")
                        nc.gpsimd.dma_gather(
                            gt[:, 0:nblk, :], src_ap,
                            idx_t[:, i0 * 8: (i0 + nblk) * 8],
                            nidx, nidx, 128, single_packet=False)
                        St = gpool.tile([128, nblk, scols], BF16, tag="S")
                        nc.vector.tensor_tensor(
                            St[:, 0:nblk, :], bcast_mid(iota_t[:], nblk),
                            col_t[:, i0:i0 + nblk].to_broadcast(
                                [128, nblk, scols]),
                            mybir.AluOpType.is_equal)
                        _emit(blocks, gt, St, psum_tile, last_pass, pi, psplit)

            def _emit(blocks, gt, St, psum_tile, last_pass, pi, psplit):
                if True:
                    for bi, (i, woff, first, last) in enumerate(blocks):
                        st = False
                        sp = last and last_pass.get(woff, pi) == pi
                        if psplit:
                            seg = psum_tile[0:DO, woff * W:(woff + 1) * W]
                            nc.tensor.matmul(
                                seg, gt[:, bi, 0:DO], St[:, bi, 0:W],
                                start=st, stop=False, skip_group_check=True)
                            nc.tensor.matmul(
                                seg, gt[:, bi, DO:2 * DO], St[:, bi, W:2 * W],
                                start=False, stop=sp, skip_group_check=True)
                        else:
                            seg = psum_tile[:, woff * W:(woff + 1) * W]
                            nc.tensor.matmul(
                                seg, gt[:, bi, :], St[:, bi, :],
                                start=st, stop=sp, skip_group_check=True)

            import os
            l1_plans = [(plan_lo, gb_lo, ilo, clo, t_xs_lo.ap(), io1, False)]
            if has_hi and not os.environ.get("GCN_NO_HI"):
                l1_plans.append((plan_hi, gb_hi, ihi, chi, t_xs_hi.ap(), io1, False))

            # ---------------- Layer 1 + transform + hs2 ----------------
            for g in range(cfg.ngrp):
                agg = psA.tile([128, G], F32, tag="agg")
                agg_layer(g, l1_plans, agg, D)
                sl = slice(g * G, (g + 1) * G)
                aggu = pool.tile([D, G], BF16, tag="aggu")
                nc.vector.tensor_tensor(aggu[:], agg[:], selfT[:, sl],
                                        mybir.AluOpType.add)
                if dbg:
                    dglt = pool.tile([D, G], F32, tag="dbgt")
                    nc.vector.tensor_copy(dglt[:], agg[:])
                    nc.sync.dma_start(t_dbg[:, sl], dglt[:])
                xf = psB.tile([DH, G], F32, tag="xf")
                nc.tensor.matmul(xf[:], W1s[:], aggu[:], start=True, stop=True)
                tm = pool.tile([DH, G], BF16, tag="tm")
                nc.vector.tensor_tensor(tm[:], xf[:], d2T[:, sl],
                                        mybir.AluOpType.mult)
                if has_b1:
                    nc.vector.tensor_tensor(tm[:], tm[:], b1dT[:, sl],
                                            mybir.AluOpType.add)
                hsg = pool.tile([DH, G], BF16, tag="hs")
                nc.scalar.activation(hsg[:], tm[:],
                                     mybir.ActivationFunctionType.Relu)
                xf2 = psB.tile([DO, G], F32, tag="xf2")
                nc.tensor.matmul(xf2[:], W2s[:], hsg[:], start=True, stop=True)
                nc.vector.tensor_copy(hs2T[:, sl], xf2[:])
                for q in range(G // 128):
                    tp = psB.tile([128, DO], BF16, tag="tp")
                    nc.tensor.transpose(
                        tp[:], hs2T[:, g * G + q * 128: g * G + (q + 1) * 128],
                        idn[:])
                    nc.scalar.activation(ccstage[:, g * (G // 128) + q, :],
                                         tp[:],
                                         mybir.ActivationFunctionType.Copy)

            # ---------------- exchange ----------------
            nfull = cfg.slice // 128
            rem = cfg.slice - nfull * 128
            flat = cc_in.ap()
            nc.sync.dma_start(
                flat[0:nfull * 128, :].rearrange("(a p) f -> p a f", p=128),
                ccstage[:, 0:nfull, :])
            if rem:
                nc.sync.dma_start(flat[nfull * 128:cfg.slice, :],
                                  ccstage[0:rem, nfull, :])
            nc.gpsimd.collective_compute(
                "AllGather", mybir.AluOpType.bypass,
                replica_groups=[list(range(cfg.n_cores))],
                ins=[cc_in.ap().opt()], outs=[cc_out.ap().opt()])

            # ---------------- Layer 2 local pass (overlaps collective) ----
            l2loc_src = cc_in.ap().rearrange("(a two) d -> a (two d)", two=2)
            for g in range(cfg.ngrp):
                aggL = psA.tile([128, G], F32, tag="agg")
                agg_layer(g, [(plan_l2l, gb_l2l, il2l, cl2l, l2loc_src, io2,
                               True)], aggL, 2 * DO)
                nc.vector.tensor_copy(locacc[:, g * G:(g + 1) * G],
                                      aggL[0:DO, :])

            # ---------------- Layer 2 remote ----------------
            l2src = cc_out.ap().rearrange("c (a two) d -> (c a) (two d)", two=2)
            for g in range(cfg.ngrp):
                agg2 = psA.tile([128, G], F32, tag="agg")
                agg_layer(g,
                          [(plan_l2r, gb_l2r, il2r, cl2r, l2src, io2, True)],
                          agg2, 2 * DO)
                sl = slice(g * G, (g + 1) * G)
                u0 = pool.tile([DO, G], BF16, tag="u0")
                nc.vector.tensor_tensor(u0[:], agg2[0:DO, :], locacc[:, sl],
                                        mybir.AluOpType.add)
                u = pool.tile([DO, G], BF16, tag="u")
                nc.vector.tensor_tensor(u[:], u0[:], hs2T[:, sl],
                                        mybir.AluOpType.add)
                u2 = pool.tile([DO, G], F32, tag="u2")
                nc.vector.tensor_tensor(u2[:], u[:], dT64[:, sl],
                                        mybir.AluOpType.mult)
                if has_b2:
                    nc.vector.tensor_scalar(
                        u2[:], u2[:], b2s[:, 0:1], None,
                        mybir.AluOpType.add)
                ncols = min(cfg.slice, (g + 1) * G) - g * G
                if ncols > 0:
                    nc.sync.dma_start(
                        t_out[:, g * G: g * G + ncols], u2[:, 0:ncols])
    nc.compile()
    return nc


def run(x, edge_index, W1, b1, W2, b2, trace=False, cfg=None):
    install_ntff_hook()
    cfg = cfg or Cfg()
    plan_lo, plan_hi, plan_l2, in_maps, has_b1, has_b2 = host_prep(
        cfg, x, edge_index, W1, b1, W2, b2)
    nc = build(cfg, plan_lo, plan_hi, plan_l2, has_b1, has_b2)
    res = run_bass_kernel_spmd(nc, in_maps, core_ids=list(range(cfg.n_cores)),
                               trace=trace)
    out = np.empty((cfg.n_nodes, cfg.d_out), np.float32)
    for c in range(cfg.n_cores):
        out[c * cfg.slice:(c + 1) * cfg.slice] = \
            np.asarray(res.results[c]["out"], np.float32).T
    return out, res


def kernel(x, edge_index, W1, b1, W2, b2):
    out, _ = run(x, edge_index, W1, b1, W2, b2, trace=False)
    return out


# revision 8
# speedup vs baseline: 1.4474x; 1.4182x over previous
"""GCN 2-layer encoder on 8 TRN2 NeuronCores via Bass/Tile.

Design (measured-primitive driven):
- Edges target-sharded across 8 cores (6250 targets/core).
- Per-edge source rows fetched with gpsimd.dma_gather (~8ns/idx,
  descriptor-bound; the only fast arbitrary-index mover).
- Aggregation: per 128-edge block, one-hot matmul on TensorE into PSUM
  [features, 512-target group]. One-hots are plain 0/1, built with one
  batched tensor_tensor is_equal per gather chunk; all dinv scaling is
  folded into epilogues / host-prescaled tables.
- L1 gathers from xs = dinv-scaled x (bf16; lo/hi split for int16 idx).
- L2 gathers 256B pair-rows of hs2 = dinv*h@W2 (bf16, AllGather'd),
  parity-split one-hot columns.
- Self-loops handled in epilogues (no gather).
"""
import numpy as np
import ml_dtypes

import concourse.bass as bass
import concourse.bacc as bacc
import concourse.mybir as mybir
import concourse.tile as tile
from concourse.bass_utils import run_bass_kernel_spmd
from concourse.library_config import mlp

BF16 = mybir.dt.bfloat16
F32 = mybir.dt.float32
I16 = mybir.dt.int16
NPBF16 = ml_dtypes.bfloat16


def install_ntff_hook():
    import sys, types
    try:
        from antenv.axon_hooks import get_axon_ntff_profile_hook  # noqa
        return
    except ImportError:
        pass
    try:
        from trn_agent_boot import trn_boot
        hook = trn_boot._ntff_profile_via_ctypes("/opt/axon/libaxon_pjrt.so")
    except Exception:
        return
    mod = types.ModuleType("antenv.axon_hooks")
    mod.get_axon_ntff_profile_hook = lambda: hook
    mod.set_axon_ntff_profile_hook = lambda h: None
    sys.modules["antenv.axon_hooks"] = mod


def bcast_mid(ap, n):
    """[P, F] AP -> [P, n, F] with stride-0 middle dim."""
    a = [list(p) for p in ap.ap]
    new = [a[0], [0, n]] + a[1:]
    return bass.AP(ap.tensor, ap.offset, new)


class Cfg:
    def __init__(self, n_nodes=50000, n_cores=8, d_in=128, d_hid=128, d_out=64,
                 win=128, grp_win=4):
        assert n_nodes % (2 * n_cores) == 0
        self.n_nodes = n_nodes
        self.n_cores = n_cores
        self.d_in = d_in
        self.d_hid = d_hid
        self.d_out = d_out
        self.slice = n_nodes // n_cores
        self.win = win
        self.grp_win = grp_win
        self.grp = win * grp_win
        self.ngrp = -(-self.slice // self.grp)
        self.nwin = self.ngrp * grp_win          # padded window count
        self.slice_pad = self.ngrp * self.grp
        self.lo_cut = min(32768, n_nodes)
        self.pairs = n_nodes // 2


def _wrap_idx(idx):
    idx = np.asarray(idx, np.int16)
    w = idx.reshape(-1, 16).T
    return np.tile(w, (8, 1)).astype(np.int16)


def _interleave_cols(vals):
    v = np.asarray(vals)
    return np.ascontiguousarray(v.reshape(-1, 128).T)


class EdgePlan:
    def __init__(self, cfg, per_core_edges, idx_of, col_of, force_min=False):
        self.cfg = cfg
        counts = np.zeros((cfg.n_cores, cfg.nwin), np.int64)
        for c in range(cfg.n_cores):
            w = per_core_edges[c]["win"]
            if len(w):
                np.add.at(counts[c], w, 1)
        bpw = -(-counts.max(axis=0) // 128)
        if force_min:
            bpw = np.maximum(bpw, 1)
        self.bpw = bpw.astype(np.int64)
        self.nblk = int(bpw.sum())
        self.idx = []
        self.col = []
        for c in range(cfg.n_cores):
            e = per_core_edges[c]
            order = np.argsort(e["win"], kind="stable")
            win_s = e["win"][order]
            idx_s = idx_of(e, order)
            col_s = col_of(e, order)
            idx_out = np.zeros(self.nblk * 128, np.int16)
            col_out = np.full(self.nblk * 128, -99.0, np.float32)
            starts = np.searchsorted(win_s, np.arange(cfg.nwin))
            ends = np.searchsorted(win_s, np.arange(cfg.nwin) + 1)
            off = 0
            for w in range(cfg.nwin):
                n = int(ends[w] - starts[w])
                cap = int(self.bpw[w]) * 128
                assert n <= cap
                idx_out[off:off + n] = idx_s[starts[w]:ends[w]]
                col_out[off:off + n] = col_s[starts[w]:ends[w]]
                off += cap
            self.idx.append(_wrap_idx(idx_out))
            self.col.append(_interleave_cols(col_out).astype(NPBF16))
        self.blocks = []
        for w in range(cfg.nwin):
            g = w // cfg.grp_win
            woff = w % cfg.grp_win
            for b in range(int(self.bpw[w])):
                self.blocks.append(
                    (g, woff, b == 0, b == int(self.bpw[w]) - 1))


def host_prep(cfg, x, edge_index, W1, b1, W2, b2):
    N = cfg.n_nodes
    src = np.asarray(edge_index[0], np.int64)
    tgt = np.asarray(edge_index[1], np.int64)
    deg = np.bincount(tgt, minlength=N).astype(np.float64) + 1.0
    dinv = (1.0 / np.sqrt(deg)).astype(np.float32)
    xs = (np.asarray(x, np.float32) * dinv[:, None]).astype(NPBF16)

    core = tgt // cfg.slice
    tloc = tgt % cfg.slice
    win = tloc // cfg.win
    ww = tloc % cfg.win

    per_core, lo, hi = [], [], []
    for c in range(cfg.n_cores):
        m = core == c
        e = {"src": src[m], "win": win[m], "ww": ww[m]}
        per_core.append(e)
        ml = e["src"] < cfg.lo_cut
        lo.append({"src": e["src"][ml], "win": e["win"][ml], "ww": e["ww"][ml]})
        mh = ~ml
        hi.append({"src": e["src"][mh] - cfg.lo_cut, "win": e["win"][mh],
                   "ww": e["ww"][mh]})

    plan_lo = EdgePlan(cfg, lo, lambda e, o: e["src"][o].astype(np.int16),
                       lambda e, o: e["ww"][o].astype(np.float32))
    plan_hi = EdgePlan(cfg, hi, lambda e, o: e["src"][o].astype(np.int16),
                       lambda e, o: e["ww"][o].astype(np.float32))
    l2loc, l2rem = [], []
    for c in range(cfg.n_cores):
        e = per_core[c]
        mloc = (e["src"] // cfg.slice) == c
        l2loc.append({
            "pidx": ((e["src"][mloc] - c * cfg.slice) >> 1).astype(np.int16),
            "win": e["win"][mloc],
            "col": (e["ww"][mloc] + cfg.win * (e["src"][mloc] & 1)).astype(np.float32),
        })
        mrem = ~mloc
        l2rem.append({
            "pidx": (e["src"][mrem] >> 1).astype(np.int16),
            "win": e["win"][mrem],
            "col": (e["ww"][mrem] + cfg.win * (e["src"][mrem] & 1)).astype(np.float32),
        })
    pick_idx = lambda e, o: e["pidx"][o]
    pick_col = lambda e, o: e["col"][o]
    plan_l2l = EdgePlan(cfg, l2loc, pick_idx, pick_col)
    plan_l2r = EdgePlan(cfg, l2rem, pick_idx, pick_col)

    shared = {
        "xs_lo": np.ascontiguousarray(xs[:cfg.lo_cut]),
        "W1": np.asarray(W1, np.float32).astype(NPBF16),
        "W2": np.asarray(W2, np.float32).astype(NPBF16),
        "iota1": np.tile(np.arange(cfg.win, dtype=np.float32),
                         (128, 1)).astype(NPBF16),
        "iota2": np.tile(np.arange(2 * cfg.win, dtype=np.float32),
                         (128, 1)).astype(NPBF16),
        "ident": np.eye(cfg.d_out, dtype=np.float32).astype(NPBF16),
    }
    if cfg.n_nodes > cfg.lo_cut:
        shared["xs_hi"] = np.ascontiguousarray(xs[cfg.lo_cut:])
    has_b1 = bool(np.any(np.asarray(b1)))
    has_b2 = bool(np.any(np.asarray(b2)))
    if has_b2:
        shared["b2col"] = np.asarray(b2, np.float32).reshape(cfg.d_out, 1)
    in_maps = []
    for c in range(cfg.n_cores):
        s0 = c * cfg.slice
        dpad = np.pad(dinv[s0:s0 + cfg.slice], (0, cfg.slice_pad - cfg.slice))
        selfT = np.zeros((cfg.d_in, cfg.slice_pad), np.float32)
        selfT[:, :cfg.slice] = xs[s0:s0 + cfg.slice].astype(np.float32).T
        m = {
            "idx_lo": plan_lo.idx[c], "col_lo": plan_lo.col[c],
            "idx_l2l": plan_l2l.idx[c], "col_l2l": plan_l2l.col[c],
            "idx_l2r": plan_l2r.idx[c], "col_l2r": plan_l2r.col[c],
            "selfT": selfT.astype(NPBF16),
            "dinv2T": np.tile(dpad * dpad, (cfg.d_hid, 1)).astype(NPBF16),
            "dinvT64": np.tile(dpad, (cfg.d_out, 1)).astype(NPBF16),
        }
        if cfg.n_nodes > cfg.lo_cut:
            m["idx_hi"] = plan_hi.idx[c]
            m["col_hi"] = plan_hi.col[c]
        if has_b1:
            m["b1dT"] = np.outer(np.asarray(b1, np.float32), dpad).astype(NPBF16)
        m.update(shared)
        in_maps.append(m)
    return plan_lo, plan_hi, (plan_l2l, plan_l2r), in_maps, has_b1, has_b2


def build(cfg, plan_lo, plan_hi, plan_l2, has_b1, has_b2):
    plan_l2l, plan_l2r = plan_l2
    nc = bacc.Bacc("TRN2", num_devices=cfg.n_cores, num_swdge_queues=4)
    D, DH, DO = cfg.d_in, cfg.d_hid, cfg.d_out
    SP = cfg.slice_pad
    G = cfg.grp
    W = cfg.win
    has_hi = cfg.n_nodes > cfg.lo_cut

    t_xs_lo = nc.dram_tensor("xs_lo", [cfg.lo_cut, D], BF16, kind="ExternalInput")
    if has_hi:
        t_xs_hi = nc.dram_tensor("xs_hi", [cfg.n_nodes - cfg.lo_cut, D], BF16,
                                 kind="ExternalInput")
    t_W1 = nc.dram_tensor("W1", [D, DH], BF16, kind="ExternalInput")
    t_W2 = nc.dram_tensor("W2", [DH, DO], BF16, kind="ExternalInput")
    t_io1 = nc.dram_tensor("iota1", [128, W], BF16, kind="ExternalInput")
    t_io2 = nc.dram_tensor("iota2", [128, 2 * W], BF16, kind="ExternalInput")
    t_id = nc.dram_tensor("ident", [DO, DO], BF16, kind="ExternalInput")
    t_selfT = nc.dram_tensor("selfT", [D, SP], BF16, kind="ExternalInput")
    t_d2T = nc.dram_tensor("dinv2T", [DH, SP], BF16, kind="ExternalInput")
    t_dT64 = nc.dram_tensor("dinvT64", [DO, SP], BF16, kind="ExternalInput")
    t_b1dT = (nc.dram_tensor("b1dT", [DH, SP], BF16, kind="ExternalInput")
              if has_b1 else None)
    t_b2 = (nc.dram_tensor("b2col", [DO, 1], F32, kind="ExternalInput")
            if has_b2 else None)
    t_ilo = nc.dram_tensor("idx_lo", list(plan_lo.idx[0].shape), I16,
                           kind="ExternalInput")
    t_clo = nc.dram_tensor("col_lo", list(plan_lo.col[0].shape), BF16,
                           kind="ExternalInput")
    if has_hi:
        t_ihi = nc.dram_tensor("idx_hi", list(plan_hi.idx[0].shape), I16,
                               kind="ExternalInput")
        t_chi = nc.dram_tensor("col_hi", list(plan_hi.col[0].shape), BF16,
                               kind="ExternalInput")
    t_il2l = nc.dram_tensor("idx_l2l", list(plan_l2l.idx[0].shape), I16,
                            kind="ExternalInput")
    t_cl2l = nc.dram_tensor("col_l2l", list(plan_l2l.col[0].shape), BF16,
                            kind="ExternalInput")
    t_il2r = nc.dram_tensor("idx_l2r", list(plan_l2r.idx[0].shape), I16,
                            kind="ExternalInput")
    t_cl2r = nc.dram_tensor("col_l2r", list(plan_l2r.col[0].shape), BF16,
                            kind="ExternalInput")
    t_out = nc.dram_tensor("out", [DO, cfg.slice], F32, kind="ExternalOutput")
    import os as _os
    dbg = bool(_os.environ.get("GCN_DBG"))
    t_dbg = (nc.dram_tensor("dbg", [D, SP], F32, kind="ExternalOutput")
             if dbg else None)

    cc_in = nc.dram_tensor("cc_in", [cfg.slice, DO], BF16)
    cc_out = nc.dram_tensor("cc_out", [cfg.n_cores, cfg.slice, DO], BF16,
                            addr_space="Shared")

    def group_blocks(plan):
        out = [[] for _ in range(cfg.ngrp)]
        for i, blk in enumerate(plan.blocks):
            out[blk[0]].append((i,) + blk[1:])
        return out

    gb_lo, gb_hi = map(group_blocks, (plan_lo, plan_hi))
    gb_l2l, gb_l2r = map(group_blocks, (plan_l2l, plan_l2r))

    with tile.TileContext(nc) as tc:
        with (
            tc.tile_pool(name="meta", bufs=1) as meta,
            tc.tile_pool(name="gath", bufs=3) as gpool,
            tc.tile_pool(name="work", bufs=3) as pool,
            tc.tile_pool(name="res", bufs=1) as res,
            tc.tile_pool(name="psA", bufs=4, space="PSUM") as psA,
            tc.tile_pool(name="psB", bufs=1, space="PSUM") as psB,
        ):
            nc.gpsimd.load_library(mlp)

            def load(t):
                s = meta.tile(list(t.shape), t.dtype, tag=f"m_{t.name}")
                nc.sync.dma_start(s[:], t[:])
                return s

            ilo, clo = load(t_ilo), load(t_clo)
            ihi, chi = (load(t_ihi), load(t_chi)) if has_hi else (None, None)
            il2l, cl2l = load(t_il2l), load(t_cl2l)
            il2r, cl2r = load(t_il2r), load(t_cl2r)
            io1, io2, idn = load(t_io1), load(t_io2), load(t_id)
            W1s, W2s = load(t_W1), load(t_W2)
            selfT, d2T, dT64 = load(t_selfT), load(t_d2T), load(t_dT64)
            b1dT = load(t_b1dT) if has_b1 else None
            b2s = load(t_b2) if has_b2 else None

            zeros = res.tile([128, cfg.grp], BF16)
            nc.gpsimd.memset(zeros[:], 0.0)
            qctr = [0]
            hs2T = res.tile([DO, SP], BF16)
            locacc = res.tile([DO, SP], BF16)
            ccstage = res.tile([128, SP // 128, DO], BF16)

            def agg_layer(g, plans, psum_tile, m_rows):
                nc.tensor.matmul(psum_tile[:], zeros[:, 0:128], zeros[:],
                                 start=True, stop=False, skip_group_check=True)
                last_pass = {}
                for pi, (plan, gb, *_r) in enumerate(plans):
                    for (i, woff, first, last) in gb[g]:
                        if last:
                            last_pass[woff] = pi
                for pi, (plan, gb, idx_t, col_t, src_ap, iota_t, psplit) \
                        in enumerate(plans):
                    all_blocks = gb[g]
                    if not all_blocks:
                        continue
                    scols = 2 * W if psplit else W
                    CH = 32
                    for c0 in range(0, len(all_blocks), CH):
                        blocks = all_blocks[c0:c0 + CH]
                        i0 = blocks[0][0]
                        nblk = len(blocks)
                        nidx = nblk * 128
                        gt = gpool.tile([128, nblk, 128], BF16, tag="g")
                        qn = qctr[0] % 4
                        qctr[0] += 1
                        nc.gpsimd.dma_gather(
                            gt[:, 0:nblk, :], src_ap,
                            idx_t[:, i0 * 8: (i0 + nblk) * 8],
                            nidx, nidx, 128, single_packet=False,
                            queue_num=qn)
                        St = gpool.tile([128, nblk, scols], BF16, tag="S")
                        nc.vector.tensor_tensor(
                            St[:, 0:nblk, :], bcast_mid(iota_t[:], nblk),
                            col_t[:, i0:i0 + nblk].to_broadcast(
                                [128, nblk, scols]),
                            mybir.AluOpType.is_equal)
                        _emit(blocks, gt, St, psum_tile, last_pass, pi, psplit)

            def _emit(blocks, gt, St, psum_tile, last_pass, pi, psplit):
                if True:
                    for bi, (i, woff, first, last) in enumerate(blocks):
                        st = False
                        sp = last and last_pass.get(woff, pi) == pi
                        if psplit:
                            seg = psum_tile[0:DO, woff * W:(woff + 1) * W]
                            nc.tensor.matmul(
                                seg, gt[:, bi, 0:DO], St[:, bi, 0:W],
                                start=st, stop=False, skip_group_check=True)
                            nc.tensor.matmul(
                                seg, gt[:, bi, DO:2 * DO], St[:, bi, W:2 * W],
                                start=False, stop=sp, skip_group_check=True)
                        else:
                            seg = psum_tile[:, woff * W:(woff + 1) * W]
                            nc.tensor.matmul(
                                seg, gt[:, bi, :], St[:, bi, :],
                                start=st, stop=sp, skip_group_check=True)

            import os
            l1_plans = [(plan_lo, gb_lo, ilo, clo, t_xs_lo.ap(), io1, False)]
            if has_hi and not os.environ.get("GCN_NO_HI"):
                l1_plans.append((plan_hi, gb_hi, ihi, chi, t_xs_hi.ap(), io1, False))

            # ---------------- Layer 1 + transform + hs2 ----------------
            for g in range(cfg.ngrp):
                agg = psA.tile([128, G], F32, tag="agg")
                agg_layer(g, l1_plans, agg, D)
                sl = slice(g * G, (g + 1) * G)
                aggu = pool.tile([D, G], BF16, tag="aggu")
                nc.vector.tensor_tensor(aggu[:], agg[:], selfT[:, sl],
                                        mybir.AluOpType.add)
                if dbg:
                    dglt = pool.tile([D, G], F32, tag="dbgt")
                    nc.vector.tensor_copy(dglt[:], agg[:])
                    nc.sync.dma_start(t_dbg[:, sl], dglt[:])
                xf = psB.tile([DH, G], F32, tag="xf")
                nc.tensor.matmul(xf[:], W1s[:], aggu[:], start=True, stop=True)
                tm = pool.tile([DH, G], BF16, tag="tm")
                nc.vector.tensor_tensor(tm[:], xf[:], d2T[:, sl],
                                        mybir.AluOpType.mult)
                if has_b1:
                    nc.vector.tensor_tensor(tm[:], tm[:], b1dT[:, sl],
                                            mybir.AluOpType.add)
                hsg = pool.tile([DH, G], BF16, tag="hs")
                nc.scalar.activation(hsg[:], tm[:],
                                     mybir.ActivationFunctionType.Relu)
                xf2 = psB.tile([DO, G], F32, tag="xf2")
                nc.tensor.matmul(xf2[:], W2s[:], hsg[:], start=True, stop=True)
                nc.vector.tensor_copy(hs2T[:, sl], xf2[:])
                for q in range(G // 128):
                    tp = psB.tile([128, DO], BF16, tag="tp")
                    nc.tensor.transpose(
                        tp[:], hs2T[:, g * G + q * 128: g * G + (q + 1) * 128],
                        idn[:])
                    nc.scalar.activation(ccstage[:, g * (G // 128) + q, :],
                                         tp[:],
                                         mybir.ActivationFunctionType.Copy)

            # ---------------- exchange ----------------
            nfull = cfg.slice // 128
            rem = cfg.slice - nfull * 128
            flat = cc_in.ap()
            nc.sync.dma_start(
                flat[0:nfull * 128, :].rearrange("(a p) f -> p a f", p=128),
                ccstage[:, 0:nfull, :])
            if rem:
                nc.sync.dma_start(flat[nfull * 128:cfg.slice, :],
                                  ccstage[0:rem, nfull, :])
            nc.gpsimd.collective_compute(
                "AllGather", mybir.AluOpType.bypass,
                replica_groups=[list(range(cfg.n_cores))],
                ins=[cc_in.ap().opt()], outs=[cc_out.ap().opt()])

            # ---------------- Layer 2 local pass (overlaps collective) ----
            l2loc_src = cc_in.ap().rearrange("(a two) d -> a (two d)", two=2)
            for g in range(cfg.ngrp):
                aggL = psA.tile([128, G], F32, tag="agg")
                agg_layer(g, [(plan_l2l, gb_l2l, il2l, cl2l, l2loc_src, io2,
                               True)], aggL, 2 * DO)
                nc.vector.tensor_copy(locacc[:, g * G:(g + 1) * G],
                                      aggL[0:DO, :])

            # ---------------- Layer 2 remote ----------------
            l2src = cc_out.ap().rearrange("c (a two) d -> (c a) (two d)", two=2)
            for g in range(cfg.ngrp):
                agg2 = psA.tile([128, G], F32, tag="agg")
                agg_layer(g,
                          [(plan_l2r, gb_l2r, il2r, cl2r, l2src, io2, True)],
                          agg2, 2 * DO)
                sl = slice(g * G, (g + 1) * G)
                u0 = pool.tile([DO, G], BF16, tag="u0")
                nc.vector.tensor_tensor(u0[:], agg2[0:DO, :], locacc[:, sl],
                                        mybir.AluOpType.add)
                u = pool.tile([DO, G], BF16, tag="u")
                nc.vector.tensor_tensor(u[:], u0[:], hs2T[:, sl],
                                        mybir.AluOpType.add)
                u2 = pool.tile([DO, G], F32, tag="u2")
                nc.vector.tensor_tensor(u2[:], u[:], dT64[:, sl],
                                        mybir.AluOpType.mult)
                if has_b2:
                    nc.vector.tensor_scalar(
                        u2[:], u2[:], b2s[:, 0:1], None,
                        mybir.AluOpType.add)
                ncols = min(cfg.slice, (g + 1) * G) - g * G
                if ncols > 0:
                    nc.sync.dma_start(
                        t_out[:, g * G: g * G + ncols], u2[:, 0:ncols])
    nc.compile()
    return nc


def run(x, edge_index, W1, b1, W2, b2, trace=False, cfg=None):
    install_ntff_hook()
    cfg = cfg or Cfg()
    plan_lo, plan_hi, plan_l2, in_maps, has_b1, has_b2 = host_prep(
        cfg, x, edge_index, W1, b1, W2, b2)
    nc = build(cfg, plan_lo, plan_hi, plan_l2, has_b1, has_b2)
    res = run_bass_kernel_spmd(nc, in_maps, core_ids=list(range(cfg.n_cores)),
                               trace=trace)
    out = np.empty((cfg.n_nodes, cfg.d_out), np.float32)
    for c in range(cfg.n_cores):
        out[c * cfg.slice:(c + 1) * cfg.slice] = \
            np.asarray(res.results[c]["out"], np.float32).T
    return out, res


def kernel(x, edge_index, W1, b1, W2, b2):
    out, _ = run(x, edge_index, W1, b1, W2, b2, trace=False)
    return out
